# revision 3
# baseline (speedup 1.0000x reference)
"""Sliding-window local attention (B=8, S=4096, D=64, W=65) on 8 trn2 cores.

Sharding: data-parallel over batch (1 batch element per core).

Per-core algorithm (all shapes per core):
  - queries tiled in 32 blocks of 128; for query block t the 65-wide window
    spans keys [t*128-32, t*128+160) -> a [128, 192] score rectangle.
  - scores = QT.T @ KT_slice with contraction K=66: rows 0..63 are the head
    dim, row 64 folds the sequence-edge bias (-1e9 outside [0,S)), row 65
    folds the key-mask bias (-1e9 where mask[j]).
  - + banded bias (-1e9 outside the per-row 65-wide window), exp on ACT with
    accumulated row sums (softmax without max-subtraction; |scores| < ~60 so
    exp can't overflow), giving unnormalized probs E with exact zeros at all
    invalid slots.
  - output: PE-transpose E, two accumulating matmuls against a 32-row-shifted
    V layout, scale by 1/rowsum.
  - probs output: E is bounced to a DRAM scratch and re-read with a
    stride-193 diagonal access pattern which extracts exactly the [128, 65]
    band, then scaled by 1/rowsum.
"""

import functools
import os

import numpy as np

B, S, D = 8, 4096, 64
HALF = 32
W = 2 * HALF + 1  # 65
PT = 128  # query tile size
NT = S // PT  # 32 query tiles
KC = 192  # key-window columns per tile (PT + 2*HALF)
KDIM = D + 2  # contraction: 64 head dims + edge-bias row + mask-bias row
KT_COLS = S + 2 * HALF  # 4160 (keys padded by HALF on both sides)
NVB = NT + 1  # V blocks in shifted layout
NEG = -1.0e9


@functools.lru_cache(maxsize=None)
def _build_program():
    import concourse.bacc as bacc
    import concourse.mybir as mybir
    import concourse.tile as tile

    f32 = mybir.dt.float32
    u8 = mybir.dt.uint8

    nc = bacc.Bacc("TRN2", target_bir_lowering=False, debug=False, num_devices=8)

    qt = nc.dram_tensor("qt", [KDIM, S], f32, kind="ExternalInput")
    kt = nc.dram_tensor("kt", [KDIM, KT_COLS], f32, kind="ExternalInput")
    vsh = nc.dram_tensor("vsh", [NVB * PT, D], f32, kind="ExternalInput")
    mask_u8 = nc.dram_tensor("mask_u8", [S], u8, kind="ExternalInput")
    bband = nc.dram_tensor("bband", [PT, KC], f32, kind="ExternalInput")
    ident = nc.dram_tensor("ident", [PT, PT], f32, kind="ExternalInput")
    out_d = nc.dram_tensor("out", [S, D], f32, kind="ExternalOutput")
    probs_d = nc.dram_tensor("probs", [S, W], f32, kind="ExternalOutput")
    # scratch for the diagonal-band extraction bounce
    e_scr = nc.dram_tensor("e_scr", [NT, PT, KC], f32)

    import concourse.bass as bass

    Exp = mybir.ActivationFunctionType.Exp

    with tile.TileContext(nc) as tc:
        with (
            tc.tile_pool(name="persist", bufs=1) as persist,
            tc.tile_pool(name="work", bufs=3) as work,
            tc.tile_pool(name="psum", bufs=2, space="PSUM") as psum,
        ):
            # ---- persistent SBUF state ----
            qt_sb = persist.tile([KDIM, S], f32)
            nc.sync.dma_start(qt_sb[:], qt.ap()[:])
            kt_sb = persist.tile([KDIM, KT_COLS], f32)
            nc.sync.dma_start(kt_sb[:], kt.ap()[:])
            v_sb = persist.tile([PT, NVB * D], f32)
            nc.sync.dma_start(
                v_sb[:].rearrange("p (b c) -> p b c", c=D),
                vsh.ap().rearrange("(b p) c -> p b c", p=PT),
            )
            bband_sb = persist.tile([PT, KC], f32)
            nc.sync.dma_start(bband_sb[:], bband.ap()[:])
            ident_sb = persist.tile([PT, PT], f32)
            nc.sync.dma_start(ident_sb[:], ident.ap()[:])

            # mask -> -1e9 * mask as f32, folded into kt row 65 (in-range cols)
            mask_sb = persist.tile([PT, S // PT], u8)
            nc.sync.dma_start(
                mask_sb[:], mask_u8.ap().rearrange("(p f) -> p f", p=PT)
            )
            maskb_sb = persist.tile([PT, S // PT], f32)
            nc.vector.tensor_copy(maskb_sb[:], mask_sb[:])
            nc.vector.tensor_scalar_mul(maskb_sb[:], maskb_sb[:], NEG)
            nc.sync.dma_start(
                kt_sb[KDIM - 1 : KDIM, HALF : HALF + S].rearrange(
                    "a (p f) -> a p f", p=PT
                ),
                maskb_sb[:],
            )

            # ---- main loop over query tiles ----
            for t in range(NT):
                ps_s = psum.tile([PT, KC], f32)
                nc.tensor.matmul(
                    ps_s[:],
                    lhsT=qt_sb[:, t * PT : (t + 1) * PT],
                    rhs=kt_sb[:, t * PT : t * PT + KC],
                    start=True,
                    stop=True,
                )

                bs = work.tile([PT, KC], f32)
                nc.vector.tensor_add(bs[:], ps_s[:], bband_sb[:])

                e_t = work.tile([PT, KC], f32)
                z_t = work.tile([PT, 1], f32)
                nc.scalar.activation(e_t[:], bs[:], Exp, accum_out=z_t[:])

                rz_t = work.tile([PT, 1], f32)
                nc.vector.reciprocal(rz_t[:], z_t[:])

                # bounce E to DRAM for the diagonal-band extraction
                nc.sync.dma_start(e_scr.ap()[t], e_t[:])

                # transpose E for the PV matmul
                ps_t1 = psum.tile([PT, PT], f32)
                nc.tensor.transpose(ps_t1[:], e_t[:, 0:PT], ident_sb[:])
                ps_t2 = psum.tile([KC - PT, PT], f32)
                nc.tensor.transpose(ps_t2[:], e_t[:, PT:KC], ident_sb[:])
                et1 = work.tile([PT, PT], f32)
                nc.scalar.copy(et1[:], ps_t1[:])
                et2 = work.tile([KC - PT, PT], f32)
                nc.scalar.copy(et2[:], ps_t2[:])

                ps_o = psum.tile([PT, D], f32)
                nc.tensor.matmul(
                    ps_o[:],
                    lhsT=et1[:],
                    rhs=v_sb[:, t * D : (t + 1) * D],
                    start=True,
                    stop=False,
                )
                nc.tensor.matmul(
                    ps_o[:],
                    lhsT=et2[:],
                    rhs=v_sb[0 : KC - PT, (t + 1) * D : (t + 2) * D],
                    start=False,
                    stop=True,
                )

                out_t = work.tile([PT, D], f32)
                nc.vector.tensor_scalar_mul(out_t[:], ps_o[:], rz_t[:, 0:1])
                nc.sync.dma_start(out_d.ap()[t * PT : (t + 1) * PT, :], out_t[:])

                # read back the 65-wide band: e_scr[t, i, i+o] at flat offset
                # t*PT*KC + i*(KC+1) + o
                packed_t = work.tile([PT, W], f32)
                nc.sync.dma_start(
                    packed_t[:],
                    bass.AP(e_scr, t * PT * KC, [[KC + 1, PT], [1, W]]),
                )
                probs_t = work.tile([PT, W], f32)
                nc.vector.tensor_scalar_mul(probs_t[:], packed_t[:], rz_t[:, 0:1])
                nc.sync.dma_start(
                    probs_d.ap()[t * PT : (t + 1) * PT, :], probs_t[:]
                )

    nc.compile()
    return nc


def _marshal_core(q_b, k_b, v_b, mask_b):
    """Build the per-core input map (host-side layout marshalling only)."""
    qt = np.empty((KDIM, S), dtype=np.float32)
    qt[0:D] = q_b.T
    qt[D:] = 1.0

    kt = np.zeros((KDIM, KT_COLS), dtype=np.float32)
    kt[0:D, HALF : HALF + S] = k_b.T
    kt[D, 0:HALF] = NEG
    kt[D, HALF + S :] = NEG
    # row D+1 (mask bias) is filled on device

    vsh = np.zeros((NVB * PT, D), dtype=np.float32)
    vsh[HALF : HALF + S] = v_b

    return {
        "qt": qt,
        "kt": kt,
        "vsh": vsh,
        "mask_u8": np.ascontiguousarray(mask_b).view(np.uint8),
    }


@functools.lru_cache(maxsize=None)
def _consts():
    i = np.arange(PT)[:, None]
    j = np.arange(KC)[None, :]
    bband = np.where((j >= i) & (j <= i + 2 * HALF), 0.0, NEG).astype(np.float32)
    ident = np.eye(PT, dtype=np.float32)
    return bband, ident


def run(query, key, value, mask, trace=False):
    from concourse.bass_utils import run_bass_kernel_spmd

    nc = _build_program()
    bband, ident = _consts()
    in_maps = []
    for b in range(B):
        m = _marshal_core(
            np.asarray(query[b], dtype=np.float32),
            np.asarray(key[b], dtype=np.float32),
            np.asarray(value[b], dtype=np.float32),
            np.asarray(mask[b]),
        )
        m["bband"] = bband
        m["ident"] = ident
        in_maps.append(m)

    res = run_bass_kernel_spmd(nc, in_maps, core_ids=list(range(B)), trace=trace)
    new_value = np.stack([res.results[b]["out"] for b in range(B)])
    probs = np.stack([res.results[b]["probs"] for b in range(B)])
    return (new_value, probs), res


def kernel(query, key, value, mask):
    (new_value, probs), _ = run(query, key, value, mask)
    return new_value, probs


# revision 4
# speedup vs baseline: 1.1409x; 1.1409x over previous
"""Sliding-window local attention (B=8, S=4096, D=64, W=65) on 8 trn2 cores.

Sharding: data-parallel over batch (1 batch element per core).

Per-core algorithm (all shapes per core):
  - queries tiled in 32 blocks of 128; for query block t the 65-wide window
    spans keys [t*128-32, t*128+160) -> a [128, 192] score rectangle.
  - scores = QT.T @ KT_slice with contraction K=66: rows 0..63 are the head
    dim, row 64 folds the sequence-edge bias (-1e9 outside [0,S)), row 65
    folds the key-mask bias (-1e9 where mask[j]).
  - + banded bias (-1e9 outside the per-row 65-wide window), exp on ACT with
    accumulated row sums (softmax without max-subtraction; |scores| < ~60 so
    exp can't overflow), giving unnormalized probs E with exact zeros at all
    invalid slots.
  - output: PE-transpose E, two accumulating matmuls against a 32-row-shifted
    V layout, scale by 1/rowsum.
  - probs output: E is bounced to a DRAM scratch and re-read with a
    stride-193 diagonal access pattern which extracts exactly the [128, 65]
    band, then scaled by 1/rowsum.
"""

import functools
import os

import numpy as np

B, S, D = 8, 4096, 64
HALF = 32
W = 2 * HALF + 1  # 65
PT = 128  # query tile size
NT = S // PT  # 32 query tiles
KC = 192  # key-window columns per tile (PT + 2*HALF)
KDIM = D + 2  # contraction: 64 head dims + edge-bias row + mask-bias row
KT_COLS = S + 2 * HALF  # 4160 (keys padded by HALF on both sides)
NVB = NT + 1  # V blocks in shifted layout
NEG = -1.0e9


@functools.lru_cache(maxsize=None)
def _build_program():
    import concourse.bacc as bacc
    import concourse.mybir as mybir
    import concourse.tile as tile

    f32 = mybir.dt.float32
    u8 = mybir.dt.uint8

    nc = bacc.Bacc("TRN2", target_bir_lowering=False, debug=False, num_devices=8)

    qt = nc.dram_tensor("qt", [KDIM, S], f32, kind="ExternalInput")
    kt = nc.dram_tensor("kt", [KDIM, KT_COLS], f32, kind="ExternalInput")
    vsh = nc.dram_tensor("vsh", [NVB * PT, D], f32, kind="ExternalInput")
    mask_u8 = nc.dram_tensor("mask_u8", [S], u8, kind="ExternalInput")
    bband = nc.dram_tensor("bband", [PT, KC], f32, kind="ExternalInput")
    ident = nc.dram_tensor("ident", [PT, PT], f32, kind="ExternalInput")
    out_d = nc.dram_tensor("out", [S, D], f32, kind="ExternalOutput")
    probs_d = nc.dram_tensor("probs", [S, W], f32, kind="ExternalOutput")
    # scratch for the diagonal-band extraction bounce
    e_scr = nc.dram_tensor("e_scr", [NT, PT, KC], f32)

    import concourse.bass as bass

    Exp = mybir.ActivationFunctionType.Exp

    with tile.TileContext(nc) as tc:
        with (
            tc.tile_pool(name="persist", bufs=1) as persist,
            tc.tile_pool(name="work", bufs=3) as work,
            tc.tile_pool(name="psum", bufs=2, space="PSUM") as psum,
        ):
            # ---- persistent SBUF state ----
            qt_sb = persist.tile([KDIM, S], f32)
            nc.sync.dma_start(qt_sb[:], qt.ap()[:])
            kt_sb = persist.tile([KDIM, KT_COLS], f32)
            nc.sync.dma_start(kt_sb[:], kt.ap()[:])
            v_sb = persist.tile([PT, NVB * D], f32)
            nc.sync.dma_start(
                v_sb[:].rearrange("p (b c) -> p b c", c=D),
                vsh.ap().rearrange("(b p) c -> p b c", p=PT),
            )
            bband_sb = persist.tile([PT, KC], f32)
            nc.sync.dma_start(bband_sb[:], bband.ap()[:])
            ident_sb = persist.tile([PT, PT], f32)
            nc.sync.dma_start(ident_sb[:], ident.ap()[:])

            # mask -> -1e9 * mask as f32, folded into kt row 65 (in-range cols)
            mask_sb = persist.tile([PT, S // PT], u8)
            nc.sync.dma_start(
                mask_sb[:], mask_u8.ap().rearrange("(p f) -> p f", p=PT)
            )
            maskb_sb = persist.tile([PT, S // PT], f32)
            nc.vector.tensor_copy(maskb_sb[:], mask_sb[:])
            nc.vector.tensor_scalar_mul(maskb_sb[:], maskb_sb[:], NEG)
            nc.sync.dma_start(
                kt_sb[KDIM - 1 : KDIM, HALF : HALF + S].rearrange(
                    "a (p f) -> a p f", p=PT
                ),
                maskb_sb[:],
            )

            # ---- main loop over query tiles ----
            for t in range(NT):
                ps_s = psum.tile([PT, KC], f32)
                nc.tensor.matmul(
                    ps_s[:],
                    lhsT=qt_sb[:, t * PT : (t + 1) * PT],
                    rhs=kt_sb[:, t * PT : t * PT + KC],
                    start=True,
                    stop=True,
                )

                bs = work.tile([PT, KC], f32)
                nc.vector.tensor_add(bs[:], ps_s[:], bband_sb[:])

                e_t = work.tile([PT, KC], f32)
                z_t = work.tile([PT, 1], f32)
                nc.scalar.activation(e_t[:], bs[:], Exp, accum_out=z_t[:])

                rz_t = work.tile([PT, 1], f32)
                nc.vector.reciprocal(rz_t[:], z_t[:])

                # bounce E to DRAM for the diagonal-band extraction
                nc.sync.dma_start(e_scr.ap()[t], e_t[:])

                # transpose E for the PV matmul
                ps_t1 = psum.tile([PT, PT], f32)
                nc.tensor.transpose(ps_t1[:], e_t[:, 0:PT], ident_sb[:])
                ps_t2 = psum.tile([KC - PT, PT], f32)
                nc.tensor.transpose(ps_t2[:], e_t[:, PT:KC], ident_sb[:])
                et1 = work.tile([PT, PT], f32)
                nc.scalar.copy(et1[:], ps_t1[:])
                et2 = work.tile([KC - PT, PT], f32)
                nc.scalar.copy(et2[:], ps_t2[:])

                ps_o = psum.tile([PT, D], f32)
                nc.tensor.matmul(
                    ps_o[:],
                    lhsT=et1[:],
                    rhs=v_sb[:, t * D : (t + 1) * D],
                    start=True,
                    stop=False,
                )
                nc.tensor.matmul(
                    ps_o[:],
                    lhsT=et2[:],
                    rhs=v_sb[0 : KC - PT, (t + 1) * D : (t + 2) * D],
                    start=False,
                    stop=True,
                )

                out_t = work.tile([PT, D], f32)
                nc.vector.tensor_scalar_mul(out_t[:], ps_o[:], rz_t[:, 0:1])
                nc.sync.dma_start(out_d.ap()[t * PT : (t + 1) * PT, :], out_t[:])

                # read back the 65-wide band: e_scr[t, i, i+o] at flat offset
                # t*PT*KC + i*(KC+1) + o
                packed_t = work.tile([PT, W], f32)
                nc.sync.dma_start(
                    packed_t[:],
                    bass.AP(e_scr, t * PT * KC, [[KC + 1, PT], [1, W]]),
                )
                probs_t = work.tile([PT, W], f32)
                nc.vector.tensor_scalar_mul(probs_t[:], packed_t[:], rz_t[:, 0:1])
                nc.sync.dma_start(
                    probs_d.ap()[t * PT : (t + 1) * PT, :], probs_t[:]
                )

    nc.compile()
    return nc


def _marshal_core(q_b, k_b, v_b, mask_b):
    """Build the per-core input map (host-side layout marshalling only)."""
    qt = np.empty((KDIM, S), dtype=np.float32)
    qt[0:D] = q_b.T
    qt[D:] = 1.0

    kt = np.zeros((KDIM, KT_COLS), dtype=np.float32)
    kt[0:D, HALF : HALF + S] = k_b.T
    kt[D, 0:HALF] = NEG
    kt[D, HALF + S :] = NEG
    # row D+1 (mask bias) is filled on device

    vsh = np.zeros((NVB * PT, D), dtype=np.float32)
    vsh[HALF : HALF + S] = v_b

    return {
        "qt": qt,
        "kt": kt,
        "vsh": vsh,
        "mask_u8": np.ascontiguousarray(mask_b).view(np.uint8),
    }


@functools.lru_cache(maxsize=None)
def _consts():
    i = np.arange(PT)[:, None]
    j = np.arange(KC)[None, :]
    bband = np.where((j >= i) & (j <= i + 2 * HALF), 0.0, NEG).astype(np.float32)
    ident = np.eye(PT, dtype=np.float32)
    return bband, ident


@functools.lru_cache(maxsize=None)
def _build_exec():
    """Build the 8-core shard_map executable ONCE; reuse across kernel() calls."""
    import jax
    import concourse.mybir as mybir
    from concourse.bass2jax import (
        _bass_exec_p,
        install_neuronx_cc_hook,
        partition_id_tensor,
    )
    from jax.sharding import Mesh, PartitionSpec
    from jax.experimental.shard_map import shard_map

    nc = _build_program()
    install_neuronx_cc_hook()
    partition_name = nc.partition_id_tensor.name if nc.partition_id_tensor else None

    in_names, out_names, out_avals, zero_outs = [], [], [], []
    for alloc in nc.m.functions[0].allocations:
        if not isinstance(alloc, mybir.MemoryLocationSet):
            continue
        if not alloc.memorylocations:
            continue
        name = alloc.memorylocations[0].name
        if alloc.kind == "ExternalInput":
            if name != partition_name:
                in_names.append(name)
        elif alloc.kind == "ExternalOutput":
            shape = tuple(alloc.tensor_shape)
            dtype = mybir.dt.np(alloc.dtype)
            out_names.append(name)
            out_avals.append(jax.core.ShapedArray(shape, dtype))
            zero_outs.append(np.zeros(shape, dtype))
    n_params = len(in_names)
    n_outs = len(out_avals)
    all_names = list(in_names) + list(out_names)
    if partition_name is not None:
        all_names.append(partition_name)
    donate = tuple(range(n_params, n_params + n_outs))

    def _body(*args):
        operands = list(args)
        if partition_name is not None:
            operands.append(partition_id_tensor())
        outs = _bass_exec_p.bind(
            *operands,
            out_avals=tuple(out_avals),
            in_names=tuple(all_names),
            out_names=tuple(out_names),
            lowering_input_output_aliases=(),
            sim_require_finite=True,
            sim_require_nnan=True,
            nc=nc,
        )
        return tuple(outs)

    devices = jax.devices()[:B]
    mesh = Mesh(np.asarray(devices), ("core",))
    in_specs = (PartitionSpec("core"),) * (n_params + n_outs)
    out_specs = (PartitionSpec("core"),) * n_outs
    sharded = jax.jit(
        shard_map(
            _body, mesh=mesh, in_specs=in_specs, out_specs=out_specs, check_rep=False
        ),
        donate_argnums=donate,
        keep_unused=True,
    )
    return sharded, in_names, out_names, out_avals, zero_outs


def _in_maps(query, key, value, mask):
    bband, ident = _consts()
    maps = []
    for b in range(B):
        m = _marshal_core(
            np.asarray(query[b], dtype=np.float32),
            np.asarray(key[b], dtype=np.float32),
            np.asarray(value[b], dtype=np.float32),
            np.asarray(mask[b]),
        )
        m["bband"] = bband
        m["ident"] = ident
        maps.append(m)
    return maps


def run(query, key, value, mask, trace=False):
    sharded, in_names, out_names, out_avals, zero_outs = _build_exec()
    maps = _in_maps(query, key, value, mask)
    concat_in = [
        np.concatenate([maps[c][n] for c in range(B)], axis=0) for n in in_names
    ]
    concat_zeros = [
        np.zeros((B * z.shape[0], *z.shape[1:]), z.dtype) for z in zero_outs
    ]
    out_arrs = sharded(*concat_in, *concat_zeros)
    res = {
        name: np.asarray(out_arrs[i]).reshape(B, *out_avals[i].shape)
        for i, name in enumerate(out_names)
    }
    return (res["out"], res["probs"]), None


def kernel(query, key, value, mask):
    (new_value, probs), _ = run(query, key, value, mask)
    return new_value, probs
